# revision 19
# baseline (speedup 1.0000x reference)
"""Trainium2 Bass kernel: cross-modal channel attention (transposed-space bf16).

Math (per batch b), with G the static [L, S] linear-interp matrix:
    qT   = img_feat[b]^T                          [S, C]  (xbar DMA-transpose load)
    tpT  = W^T-matmul: tpT[c,l] = sum_d W[d,c] txt[l,d]   [C, L]
    tp   = tpT^T (PE transpose)                   [L, C]
    GQT  = G @ qT                                 [L, C]
    E^T  = exp(tp^T @ GQT * S^-0.5)               [Cj, Ci]
    EP   = E @ [tp^T | 1]  (Z from ones column)   [Ci, L+1]
    EPs  = EP / Z ; EPsT = EPs^T                  [L, Ci]
    outT = qT + (gamma*G)^T @ EPsT                [S, C]  (host transposes back)

Sharding: data-parallel over batch across 8 cores (4 batches/core);
weights/interp matrices replicated.  All matmul operands bf16 (fp32 PSUM
accumulation); I/O in bf16 to halve HBM traffic; final transpose of the
output back to [C, S] happens on host (pure layout, no math).
"""

import sys

sys.path.insert(0, "/opt/trn_rl_repo")

from contextlib import ExitStack

import ml_dtypes
import numpy as np

import concourse.bacc as bacc
import concourse.mybir as mybir
import concourse.tile as tile
from concourse.bass_utils import run_bass_kernel_spmd
from concourse.masks import make_identity

B, C, HH, WW = 32, 768, 32, 32
S = HH * WW
L, D = 77, 512
LP = 80  # L padded (zero rows) for xbar/tile alignment
N_CORES = 8
B_CORE = B // N_CORES
P = 128
CT, ST, DT = C // P, S // P, D // P
F32 = mybir.dt.float32
BF16 = mybir.dt.bfloat16
SCALE = float(S) ** -0.5
EXP = mybir.ActivationFunctionType.Exp
ADD = mybir.AluOpType.add
BF = ml_dtypes.bfloat16
HC = C // 2  # half of C, for 1-bank psum tiles


def _interp_matrix():
    """G[l, s] such that (tp^T @ G)[c, s] == linear_interp(tp^T, S)[c, s]."""
    src = np.clip(
        (np.arange(S, dtype=np.float32) + np.float32(0.5)) * np.float32(L / S)
        - np.float32(0.5),
        np.float32(0.0),
        np.float32(L - 1),
    )
    i0 = np.floor(src).astype(np.int32)
    i1 = np.minimum(i0 + 1, L - 1)
    w = (src - i0.astype(np.float32)).astype(np.float32)
    g = np.zeros((L, S), dtype=np.float32)
    g[i0, np.arange(S)] += np.float32(1.0) - w
    g[i1, np.arange(S)] += w
    return g


def _build():
    nc = bacc.Bacc("TRN2", target_bir_lowering=False, debug=False)
    img = nc.dram_tensor("img", [B_CORE, C, S], BF16, kind="ExternalInput").ap()
    # text^T, host-prearranged: txtt[p, k, b, l] = text[b, l, k*128+p]
    txtt = nc.dram_tensor("txtt", [P, DT, B_CORE, LP], BF16, kind="ExternalInput").ap()
    wt = nc.dram_tensor("wt", [P, DT, C], BF16, kind="ExternalInput").ap()
    gg = nc.dram_tensor("gg", [LP, S], BF16, kind="ExternalInput").ap()  # gamma*G
    gt = nc.dram_tensor("gt", [P, ST, LP], BF16, kind="ExternalInput").ap()  # G^T
    out = nc.dram_tensor("out", [B_CORE, S, C], BF16, kind="ExternalOutput").ap()

    with ExitStack() as ctx:
        tc = ctx.enter_context(tile.TileContext(nc))
        consts = ctx.enter_context(tc.tile_pool(name="consts", bufs=1))
        qt_pool = ctx.enter_context(tc.tile_pool(name="qt", bufs=2))
        small = ctx.enter_context(tc.tile_pool(name="small", bufs=2))
        et_pool = ctx.enter_context(tc.tile_pool(name="et", bufs=2))
        outp = ctx.enter_context(tc.tile_pool(name="outp", bufs=2))
        zp = ctx.enter_context(tc.tile_pool(name="zp", bufs=3))
        # PSUM budget (8 banks): ph1 1 + ph3 1 + gqtA 1 + gqtB 1 + psl 2x1 + pso 2x1
        ps = ctx.enter_context(tc.tile_pool(name="ps", bufs=1, space="PSUM"))

        w_sb = consts.tile([P, DT, C], BF16)
        nc.sync.dma_start(w_sb[:], wt)
        gg_sb = consts.tile([P, S], BF16)
        nc.sync.dma_start(gg_sb[0:LP, :], gg)
        gt_sb = consts.tile([P, ST, LP], BF16)
        nc.sync.dma_start(gt_sb[:], gt)
        txtt_sb = consts.tile([P, DT, B_CORE, LP], BF16)
        nc.sync.dma_start(txtt_sb[:], txtt)
        ident_f = consts.tile([P, P], F32)
        make_identity(nc, ident_f[:])
        ident = consts.tile([P, P], BF16)
        nc.vector.tensor_copy(ident[:], ident_f[:])

        for b in range(B_CORE):
            # qT tiles via xbar DMA transpose (compact 2D outputs required)
            qts = []
            for st in range(ST):
                q = qt_pool.tile([P, C], BF16, tag=f"qt{st}")
                nc.sync.dma_start_transpose(q[:], img[b][:, st * P : (st + 1) * P])
                qts.append(q)

            # GQT = G @ qT  [LP, C] accumulated over s-tiles (fp32 psum)
            ps_gqtA = ps.tile([P, HC], F32, tag="gqtA")
            ps_gqtB = ps.tile([P, HC], F32, tag="gqtB")
            for st in range(ST):
                nc.tensor.matmul(
                    ps_gqtA[0:LP, :],
                    gt_sb[:, st, :],
                    qts[st][:, 0:HC],
                    start=(st == 0),
                    stop=(st == ST - 1),
                )
                nc.tensor.matmul(
                    ps_gqtB[0:LP, :],
                    gt_sb[:, st, :],
                    qts[st][:, HC:C],
                    start=(st == 0),
                    stop=(st == ST - 1),
                )
            gqt_sb = small.tile([P, C], BF16, tag="gqt")
            nc.scalar.copy(gqt_sb[0:LP, 0:HC], ps_gqtA[0:LP, :])
            nc.scalar.copy(gqt_sb[0:LP, HC:C], ps_gqtB[0:LP, :])

            # tpT[c, l] = sum_d W[d, c] txt[l, d]   [C, LP] (fp32 psum)
            ps_tpT = ps.tile([P, CT, LP + 1], F32, tag="phX", bufs=2)
            for k in range(DT):
                for ct in range(CT):
                    nc.tensor.matmul(
                        ps_tpT[:, ct, 0:LP],
                        w_sb[:, k, ct * P : (ct + 1) * P],
                        txtt_sb[:, k, b, :],
                        start=(k == 0),
                        stop=(k == DT - 1),
                    )
            # tpT_sb has LP+1 columns; col LP holds ones (Z accumulator column)
            tpT_sb = small.tile([P, CT, LP + 1], BF16, tag="tpT")
            nc.scalar.copy(tpT_sb[:, :, 0:LP], ps_tpT[:, :, 0:LP])
            nc.gpsimd.memset(
                tpT_sb[:, :, LP : LP + 1].rearrange("p a b -> p (a b)"), 1.0
            )

            # tp = tpT^T  [LP, C] via PE transposes (bf16 psum)
            ps_tp = ps.tile([P, C], BF16, tag="phX", bufs=2)
            for jt in range(CT):
                nc.tensor.transpose(
                    ps_tp[0:LP, jt * P : (jt + 1) * P],
                    tpT_sb[:, jt, 0:LP],
                    ident[:],
                )
            tp_sb = small.tile([P, C], BF16, tag="tp")
            nc.scalar.copy(tp_sb[0:LP, :], ps_tp[0:LP, :])

            # logits^T per j-tile (half-width psums) + fused exp -> E^T (bf16)
            et_sb = et_pool.tile([P, CT, C], BF16, tag="et")
            for jt in range(CT):
                lhsT = tp_sb[0:LP, jt * P : (jt + 1) * P]
                psl_a = ps.tile([P, HC], F32, tag="psl", bufs=2)
                nc.tensor.matmul(
                    psl_a[:], lhsT, gqt_sb[0:LP, 0:HC], start=True, stop=True
                )
                nc.scalar.activation(et_sb[:, jt, 0:HC], psl_a[:], EXP, scale=SCALE)
                psl_b = ps.tile([P, HC], F32, tag="psl", bufs=2)
                nc.tensor.matmul(
                    psl_b[:], lhsT, gqt_sb[0:LP, HC:C], start=True, stop=True
                )
                nc.scalar.activation(et_sb[:, jt, HC:C], psl_b[:], EXP, scale=SCALE)

            # EP = E @ [tp^T | 1]  [Ci, LP+1]; col LP = Z_i  (fp32 psum)
            ps_ep = ps.tile([P, CT, LP + 1], F32, tag="phX", bufs=2)
            for jt in range(CT):
                for it in range(CT):
                    nc.tensor.matmul(
                        ps_ep[:, it, :],
                        et_sb[:, jt, it * P : (it + 1) * P],
                        tpT_sb[:, jt, :],
                        start=(jt == 0),
                        stop=(jt == CT - 1),
                    )
            # EPs = EP / Z  (bf16)
            eps_sb = small.tile([P, CT, LP], BF16, tag="eps")
            rz = zp.tile([P, CT], F32, tag="rz")
            nc.vector.reciprocal(
                rz[:], ps_ep[:, :, LP : LP + 1].rearrange("p a b -> p (a b)")
            )
            for it in range(CT):
                nc.vector.tensor_scalar_mul(
                    eps_sb[:, it, :], ps_ep[:, it, 0:LP], rz[:, it : it + 1]
                )

            # EPsT = EPs^T  [LP, C] via PE transposes (bf16 psum)
            ps_epsT = ps.tile([P, C], BF16, tag="phX", bufs=2)
            for it in range(CT):
                nc.tensor.transpose(
                    ps_epsT[0:LP, it * P : (it + 1) * P],
                    eps_sb[:, it, :],
                    ident[:],
                )
            epsT_sb = small.tile([P, C], BF16, tag="epsT")
            nc.scalar.copy(epsT_sb[0:LP, :], ps_epsT[0:LP, :])

            # outT = qT + (gamma*G)^T @ EPsT  per s-tile halves; resid on DVE
            o_sb = outp.tile([P, ST, C], BF16, tag="o")
            for st in range(ST):
                lhsT = gg_sb[0:LP, st * P : (st + 1) * P]
                for h in range(2):
                    pso = ps.tile([P, HC], F32, tag="pso", bufs=2)
                    nc.tensor.matmul(
                        pso[:],
                        lhsT,
                        epsT_sb[0:LP, h * HC : (h + 1) * HC],
                        start=True,
                        stop=True,
                    )
                    nc.vector.tensor_tensor(
                        o_sb[:, st, h * HC : (h + 1) * HC],
                        pso[:],
                        qts[st][:, h * HC : (h + 1) * HC],
                        ADD,
                    )
            nc.scalar.dma_start(out[b].rearrange("(st p) c -> p st c", p=P), o_sb[:])

    nc.compile()
    return nc


_NC = None


def _get_nc():
    global _NC
    if _NC is None:
        _NC = _build()
    return _NC


def _in_maps(img_feat, text_feat, W_txt, gamma):
    img = np.ascontiguousarray(
        img_feat.reshape(B, C, S), dtype=np.float32
    ).astype(BF)
    # txtt[p, k, b, l] = text[b, l, k*128+p]
    txtt = np.zeros((P, DT, B, LP), dtype=BF)
    t = np.asarray(text_feat, dtype=np.float32).astype(BF)  # [B, L, D]
    txtt[:, :, :, 0:L] = t.transpose(2, 0, 1).reshape(DT, P, B, L).transpose(1, 0, 2, 3)
    wt = np.ascontiguousarray(
        np.asarray(W_txt, dtype=np.float32).reshape(DT, P, C).transpose(1, 0, 2)
    ).astype(BF)
    g = _interp_matrix()
    gam = np.float32(np.asarray(gamma).reshape(-1)[0])
    gg = np.zeros((LP, S), dtype=BF)
    gg[0:L] = (gam * g).astype(BF)
    gt = np.zeros((P, ST, LP), dtype=BF)
    gt[:, :, 0:L] = g.T.reshape(ST, P, L).transpose(1, 0, 2).astype(BF)
    maps = []
    for m in range(N_CORES):
        sl = slice(m * B_CORE, (m + 1) * B_CORE)
        maps.append(
            {
                "img": np.ascontiguousarray(img[sl]),
                "txtt": np.ascontiguousarray(txtt[:, :, sl]),
                "wt": wt,
                "gg": gg,
                "gt": gt,
            }
        )
    return maps


def _run(in_maps, **kwargs):
    nc = _get_nc()
    return run_bass_kernel_spmd(nc, in_maps, core_ids=list(range(N_CORES)), **kwargs)


def kernel(img_feat, text_feat, W_txt, gamma):
    res = _run(_in_maps(img_feat, text_feat, W_txt, gamma))
    full = np.concatenate(
        [np.asarray(res.results[m]["out"]) for m in range(N_CORES)], axis=0
    )  # [B, S, C] bf16
    full = full.astype(np.float32).transpose(0, 2, 1)
    return np.ascontiguousarray(full.reshape(B, C, HH, WW), dtype=np.float32)
